# revision 4
# baseline (speedup 1.0000x reference)
import sys
sys.path.insert(0, '/opt/trn_rl_repo')
import numpy as np

N_USERS = 150000
N_ITEMS = 50000
N = 200000
D = 64
F = 4
d = 16
HID = 64
NC = 8
SLOTS = 25088          # per-core slots (196 windows * 128)
W = 196                # dest windows per core
CAP = 384              # gather slots per (block, window): 3 chunks of 128
CN = CAP // 128        # chunks per (block, window) = 3
WPC = 2                # windows per gather call
CALL = WPC * CAP       # idxs per gather call per block = 768
S16 = CALL // 16
O = W // WPC           # 98 call-steps
NNZ = 3200000


def _build_and_run(inputs):
    import concourse.bass as bass
    import concourse.bacc as bacc
    import concourse.mybir as mybir
    import concourse.tile as tile
    from concourse.bass_utils import run_bass_kernel_spmd
    from concourse.masks import make_identity
    from concourse._compat import get_trn_type
    import jax

    user_table = np.asarray(inputs['user_table'], np.float32)
    item_table = np.asarray(inputs['item_table'], np.float32)
    rows = np.asarray(inputs['rows'], np.int64)
    cols = np.asarray(inputs['cols'], np.int64)
    vals = np.asarray(inputs['vals'], np.float32)
    gates = np.asarray(inputs['gates'], np.float32)

    emb = np.concatenate([user_table, item_table], axis=0)  # [N, 64]

    # ---------- host: node -> (core, slot) ----------
    deg = np.bincount(rows, minlength=N)
    order = np.argsort(-deg, kind='stable')
    snake = np.concatenate([np.arange(NC), np.arange(NC)[::-1]])
    core_of = np.empty(N, np.int32)
    core_of[order] = snake[np.arange(N) % (2 * NC)]
    slot_of = np.empty(N, np.int32)
    srng = np.random.default_rng(12345)
    for k in range(NC):
        m = order[core_of[order] == k]
        slot_of[m] = srng.permutation(SLOTS)[:len(m)].astype(np.int32)

    # ---------- host: edge grouping ----------
    dcore = core_of[rows]
    dslot = slot_of[rows]
    wi = dslot // 128
    destloc = (dslot % 128).astype(np.float32)
    bb = core_of[cols]
    sidx = slot_of[cols].astype(np.int16)

    gkey = (dcore.astype(np.int64) * NC + bb) * W + wi
    NG = NC * NC * W
    cnt = np.bincount(gkey, minlength=NG)
    assert cnt.max() <= CAP, int(cnt.max())
    es = np.argsort(gkey, kind='stable')
    starts = np.zeros(NG + 1, np.int64)
    starts[1:] = np.cumsum(cnt)
    rank = np.arange(NNZ, dtype=np.int64) - starts[gkey[es]]

    # per-core padded streams
    idx_stream = np.zeros((NC, NC, W * CAP), np.int16)   # [dcore, b, w*CAP+rank]
    meta_dl = np.zeros((NC, 128, W * CN * NC), np.float32)
    meta_vv = np.zeros((NC, 128, W * CN * NC), np.float32)
    e_dc = dcore[es]; e_b = bb[es]; e_w = wi[es]
    idx_stream[e_dc, e_b, e_w * CAP + rank] = sidx[es]
    ccol = e_w * (CN * NC) + e_b * CN + rank // 128
    meta_dl[e_dc, rank % 128, ccol] = destloc[es]
    meta_vv[e_dc, rank % 128, ccol] = vals[es]

    # wrap idx for dma_gather: [NC cores][NC blocks][O calls][128][S16]
    st = idx_stream.reshape(NC, NC, O, S16, 16).transpose(0, 1, 2, 4, 3)  # [.,.,O,16,S16]
    idx_in = np.ascontiguousarray(np.tile(st[:, :, :, None, :, :], (1, 1, 1, 8, 1, 1))
                                  .reshape(NC, NC, O, 128, S16))

    # ---------- host: noise (must match jax reference) ----------
    cpu = jax.devices('cpu')[0]
    with jax.default_device(cpu):
        nk = jax.random.key(42)
        nz = []
        for l in range(2):
            kc = jax.random.fold_in(nk, 2 * l)
            ks = jax.random.fold_in(nk, 2 * l + 1)
            nz.append((np.asarray(jax.random.normal(kc, (N, F, d), np.float32)),
                       np.asarray(jax.random.normal(ks, (N, F, d), np.float32))))
    g_l = [1.0 / (1.0 + np.exp(-gates[l].astype(np.float64))) for l in range(2)]
    g_l = [g.astype(np.float32).reshape(64) for g in g_l]

    # noise in slot space, feature-major, prescaled: [core][layer][branch][64][SLOTS]
    nz_in = np.zeros((NC, 2, 2, 64, SLOTS), np.float32)
    for l in range(2):
        sc = [g_l[l], 1.0 - g_l[l]]
        for br in range(2):
            a = nz[l][br].reshape(N, 64) * sc[br][None, :]
            tmp = np.zeros((NC, SLOTS, 64), np.float32)
            tmp[core_of, slot_of] = a
            nz_in[:, l, br] = tmp.transpose(0, 2, 1)

    # ---------- host: weights ----------
    def w1_perm(W1):
        out = np.zeros((128, 256), np.float32)
        for f in range(F):
            out[16 * f:16 * f + 16, 64 * f:64 * f + 64] = W1[f, :16]
            out[64 + 16 * f:64 + 16 * f + 16, 64 * f:64 * f + 64] = W1[f, 16:]
        return out

    def w2_parts(W2, scale_mu):
        # returns mu halves [2][128,64] (g-scaled) and lv halves [2][128,64]
        mus, lvs = [], []
        for half in range(2):
            mu = np.zeros((128, 64), np.float32)
            lv = np.zeros((128, 64), np.float32)
            for fo in range(2):
                f = half * 2 + fo
                mu[64 * fo:64 * fo + 64, 16 * f:16 * f + 16] = \
                    W2[f, :, :16] * scale_mu[16 * f:16 * f + 16][None, :]
                lv[64 * fo:64 * fo + 64, 16 * f:16 * f + 16] = W2[f, :, 16:]
            mus.append(mu); lvs.append(lv)
        return mus, lvs

    Wc1, bc1 = np.asarray(inputs['Wc1'], np.float32), np.asarray(inputs['bc1'], np.float32)
    Wc2, bc2 = np.asarray(inputs['Wc2'], np.float32), np.asarray(inputs['bc2'], np.float32)
    Ws1, bs1 = np.asarray(inputs['Ws1'], np.float32), np.asarray(inputs['bs1'], np.float32)
    Ws2, bs2 = np.asarray(inputs['Ws2'], np.float32), np.asarray(inputs['bs2'], np.float32)

    w1_in = np.stack([w1_perm(Wc1), w1_perm(Ws1)])            # [2, 128, 256]
    b1_in = np.zeros((2, 2, 128), np.float32)                 # [branch][half][128]
    for br, b1 in enumerate([bc1, bs1]):
        for half in range(2):
            for fo in range(2):
                b1_in[br, half, 64 * fo:64 * fo + 64] = b1[half * 2 + fo]
    # mu/lv weight tensors per layer
    w2mu_in = np.zeros((2, 4, 128, 64), np.float32)   # [layer][c01,c23,s01,s23]
    w2lv_in = np.zeros((4, 128, 64), np.float32)      # [c01,c23,s01,s23]
    bmu_in = np.zeros((2, 64), np.float32)
    blv_in = np.zeros((2, 64), np.float32)            # [branch][64] (0.5x prescaled)
    bc2mu = bc2[:, :16].reshape(64); bs2mu = bs2[:, :16].reshape(64)
    blv_in[0] = 0.5 * bc2[:, 16:].reshape(64)
    blv_in[1] = 0.5 * bs2[:, 16:].reshape(64)
    for l in range(2):
        g = g_l[l]
        mc, _ = w2_parts(Wc2, g)
        ms, _ = w2_parts(Ws2, 1.0 - g)
        w2mu_in[l] = np.stack([mc[0], mc[1], ms[0], ms[1]])
        bmu_in[l] = g * bc2mu + (1.0 - g) * bs2mu
    _, lc = w2_parts(Wc2, np.ones(64, np.float32))
    _, ls = w2_parts(Ws2, np.ones(64, np.float32))
    w2lv_in[:] = np.stack([lc[0], lc[1], ls[0], ls[1]])

    emb_all = np.zeros((NC * SLOTS, D), np.float32)
    emb_all.reshape(NC, SLOTS, D)[core_of, slot_of] = emb
    iota_np = np.tile(np.arange(128, dtype=np.float32)[None, :], (128, 1))

    # ---------- build device program ----------
    nc = bacc.Bacc(get_trn_type() or "TRN2", debug=False, num_devices=NC)
    dt = mybir.dt
    t_emb_all = nc.dram_tensor("emb_all", [NC * SLOTS, D], dt.float32, kind="ExternalInput")
    t_emb_my = nc.dram_tensor("emb_my", [SLOTS, D], dt.float32, kind="ExternalInput")
    t_idx = [nc.dram_tensor(f"idx{l}", [NC, O, 128, S16], dt.int16, kind="ExternalInput") for l in range(2)]
    t_mdl = [nc.dram_tensor(f"mdl{l}", [128, W * CN * NC], dt.float32, kind="ExternalInput") for l in range(2)]
    t_mvv = [nc.dram_tensor(f"mvv{l}", [128, W * CN * NC], dt.float32, kind="ExternalInput") for l in range(2)]
    t_nz = nc.dram_tensor("nz", [2, 2, 64, SLOTS], dt.float32, kind="ExternalInput")
    t_w1 = nc.dram_tensor("w1", [2, 128, 256], dt.float32, kind="ExternalInput")
    t_b1 = nc.dram_tensor("b1", [2, 2, 128], dt.float32, kind="ExternalInput")
    t_w2mu = nc.dram_tensor("w2mu", [2, 4, 128, 64], dt.float32, kind="ExternalInput")
    t_w2lv = nc.dram_tensor("w2lv", [4, 128, 64], dt.float32, kind="ExternalInput")
    t_bmu = nc.dram_tensor("bmu", [2, 64], dt.float32, kind="ExternalInput")
    t_blv = nc.dram_tensor("blv", [2, 64], dt.float32, kind="ExternalInput")
    t_iota = nc.dram_tensor("iota", [128, 128], dt.float32, kind="ExternalInput")
    t_out = nc.dram_tensor("outp", [SLOTS, D], dt.float32, kind="ExternalOutput")

    CNB = CN * NC  # chunk cols per window = 24

    with tile.TileContext(nc) as tc:
        cur_slice, _f1 = tc.tile([SLOTS, D], dt.float32, space="DRAM", name="cur_slice")
        cur_full, _f2 = tc.tile([NC * SLOTS, D], dt.float32, space="DRAM",
                                addr_space="Shared", name="cur_full")
        with tc.tile_pool(name="cst", bufs=1) as cst, \
             tc.tile_pool(name="big", bufs=1) as bigp, \
             tc.tile_pool(name="sp", bufs=2) as sp, \
             tc.tile_pool(name="mt", bufs=2) as mt, \
             tc.tile_pool(name="mlp", bufs=2) as ml, \
             tc.tile_pool(name="pw", bufs=1, space="PSUM") as pwp, \
             tc.tile_pool(name="pc", bufs=1, space="PSUM") as pcp, \
             tc.tile_pool(name="ph", bufs=2, space="PSUM") as php, \
             tc.tile_pool(name="pa", bufs=1, space="PSUM") as pap:
            iota_t = cst.tile([128, 128], dt.float32)
            nc.sync.dma_start(out=iota_t[:], in_=t_iota[:, :])
            ident = cst.tile([128, 128], dt.float32)
            make_identity(nc, ident[:])
            w1_t = cst.tile([128, 2, 256], dt.float32)
            nc.sync.dma_start(out=w1_t[:], in_=t_w1[:, :, :].rearrange("b p h -> p b h"))
            b1_t = cst.tile([128, 2 * 2], dt.float32)
            nc.sync.dma_start(out=b1_t[:], in_=t_b1[:, :, :].rearrange("a b p -> p (a b)"))
            w2mu_t = cst.tile([128, 2, 4, 64], dt.float32)
            nc.sync.dma_start(out=w2mu_t[:], in_=t_w2mu[:, :, :, :].rearrange("l j p h -> p l j h"))
            w2lv_t = cst.tile([128, 4, 64], dt.float32)
            nc.sync.dma_start(out=w2lv_t[:], in_=t_w2lv[:, :, :].rearrange("j p h -> p j h"))
            bmu_t = cst.tile([64, 2], dt.float32)
            nc.sync.dma_start(out=bmu_t[:], in_=t_bmu[:, :].rearrange("l p -> p l"))
            blv_t = cst.tile([64, 2], dt.float32)
            nc.sync.dma_start(out=blv_t[:], in_=t_blv[:, :].rearrange("b p -> p b"))
            bigtile = bigp.tile([128, W, 128], dt.float32)

            # layer-1 cur = emb_my into bigtile[:, :, 0:64]
            nc.sync.dma_start(
                out=bigtile[:, :, 0:64],
                in_=t_emb_my[:, :].rearrange("(w p) d -> p w d", p=128))

            for l in range(2):
                src = t_emb_all if l == 0 else cur_full
                # ---- SpMV ----
                for o in range(O):
                    dl_t = mt.tile([128, WPC * CNB], dt.float32, tag="dl")
                    vv_t = mt.tile([128, WPC * CNB], dt.float32, tag="vv")
                    nc.sync.dma_start(out=dl_t[:], in_=t_mdl[l][:, o * WPC * CNB:(o + 1) * WPC * CNB])
                    nc.sync.dma_start(out=vv_t[:], in_=t_mvv[l][:, o * WPC * CNB:(o + 1) * WPC * CNB])
                    temps = []
                    for b in range(NC):
                        idx_t = sp.tile([128, S16], dt.int16, tag=f"ix{b}")
                        nc.sync.dma_start(out=idx_t[:], in_=t_idx[l][b, o, :, :])
                        tmp = sp.tile([128, CALL // 128, D], dt.float32, tag=f"tp{b}")
                        nc.gpsimd.dma_gather(tmp[:], src[b * SLOTS:(b + 1) * SLOTS, :],
                                             idx_t[:], CALL, CALL, D, single_packet=False)
                        temps.append(tmp)
                    for wo in range(WPC):
                        w = o * WPC + wo
                        pw = pwp.tile([128, D], dt.float32, space="PSUM", tag="pw")
                        for b in range(NC):
                            for i in range(CN):
                                cc = wo * CNB + b * CN + i
                                M = sp.tile([128, 128], dt.float32, tag="M")
                                nc.vector.tensor_scalar(
                                    out=M[:], in0=iota_t[:],
                                    scalar1=dl_t[:, cc:cc + 1],
                                    scalar2=vv_t[:, cc:cc + 1],
                                    op0=mybir.AluOpType.is_equal,
                                    op1=mybir.AluOpType.mult)
                                nc.tensor.matmul(
                                    out=pw[:], lhsT=M[:],
                                    rhs=temps[b][:, wo * CN + i, :],
                                    start=(b == 0 and i == 0),
                                    stop=(b == NC - 1 and i == CN - 1))
                        nc.vector.tensor_copy(out=bigtile[:, w, 64:128], in_=pw[:])
                # ---- MLP ---- (groups of 4 windows = 512 nodes)
                for gidx in range(W // 4):
                    comb = ml.tile([128, 512], dt.float32, tag="comb")
                    for wo in range(4):
                        w = gidx * 4 + wo
                        pc = pcp.tile([128, 128], dt.float32, space="PSUM", tag="pc")
                        nc.tensor.transpose(out=pc[:], in_=bigtile[:, w, :], identity=ident[:])
                        nc.scalar.copy(out=comb[:, wo * 128:(wo + 1) * 128], in_=pc[:])
                    hs = []
                    for br in range(2):
                        for half in range(2):
                            ph = php.tile([128, 512], dt.float32, space="PSUM", tag="ph")
                            nc.tensor.matmul(out=ph[:], lhsT=w1_t[:, br, half * 128:(half + 1) * 128],
                                             rhs=comb[:], start=True, stop=True)
                            h = ml.tile([128, 512], dt.float32, tag=f"h{br}{half}")
                            nc.scalar.activation(h[:], ph[:], mybir.ActivationFunctionType.Relu,
                                                 bias=b1_t[:, br * 2 + half:br * 2 + half + 1], scale=1.0)
                            hs.append(h)
                    # A_mu
                    pA = pap.tile([64, 512], dt.float32, space="PSUM", tag="pA")
                    for j in range(4):  # c01,c23,s01,s23
                        nc.tensor.matmul(out=pA[:], lhsT=w2mu_t[:, l, j, :], rhs=hs[j][:],
                                         start=(j == 0), stop=(j == 3))
                    # lv per branch
                    Es = []
                    for br in range(2):
                        pL = pap.tile([64, 512], dt.float32, space="PSUM", tag=f"pL{br}")
                        nc.tensor.matmul(out=pL[:], lhsT=w2lv_t[:, 2 * br, :], rhs=hs[2 * br][:],
                                         start=True, stop=False)
                        nc.tensor.matmul(out=pL[:], lhsT=w2lv_t[:, 2 * br + 1, :], rhs=hs[2 * br + 1][:],
                                         start=False, stop=True)
                        E = ml.tile([64, 512], dt.float32, tag=f"E{br}")
                        nc.scalar.activation(E[:], pL[:], mybir.ActivationFunctionType.Exp,
                                             bias=blv_t[:, br:br + 1], scale=0.5)
                        Es.append(E)
                    Ff = ml.tile([64, 512], dt.float32, tag="Ff")
                    nc.scalar.activation(Ff[:], pA[:], mybir.ActivationFunctionType.Identity,
                                         bias=bmu_t[:, l:l + 1], scale=1.0)
                    for br in range(2):
                        nzt = ml.tile([64, 512], dt.float32, tag=f"nz{br}")
                        nc.sync.dma_start(out=nzt[:], in_=t_nz[l, br, :, gidx * 512:(gidx + 1) * 512])
                        tt = ml.tile([64, 512], dt.float32, tag=f"tt{br}")
                        nc.vector.tensor_tensor(out=tt[:], in0=nzt[:], in1=Es[br][:],
                                                op=mybir.AluOpType.mult)
                        nc.vector.tensor_tensor(out=Ff[:], in0=Ff[:], in1=tt[:],
                                                op=mybir.AluOpType.add)
                    # back to node-major
                    for wo in range(4):
                        w = gidx * 4 + wo
                        pF = pcp.tile([128, 64], dt.float32, space="PSUM", tag="pF")
                        nc.tensor.transpose(out=pF[:], in_=Ff[:, wo * 128:(wo + 1) * 128],
                                            identity=ident[0:64, 0:64])
                        if l == 0:
                            nc.vector.tensor_copy(out=bigtile[:, w, 0:64], in_=pF[:])
                            nc.sync.dma_start(
                                out=cur_slice[w * 128:(w + 1) * 128, :],
                                in_=bigtile[:, w, 0:64])
                        else:
                            et = ml.tile([128, 64], dt.float32, tag="et")
                            nc.sync.dma_start(out=et[:], in_=t_emb_my[w * 128:(w + 1) * 128, :])
                            s1 = ml.tile([128, 64], dt.float32, tag="s1")
                            nc.vector.tensor_tensor(out=s1[:], in0=et[:], in1=bigtile[:, w, 0:64],
                                                    op=mybir.AluOpType.add)
                            nc.vector.tensor_tensor(out=s1[:], in0=s1[:], in1=pF[:],
                                                    op=mybir.AluOpType.add)
                            of = ml.tile([128, 64], dt.float32, tag="of")
                            nc.scalar.mul(out=of[:], in_=s1[:], mul=1.0 / 3.0)
                            nc.sync.dma_start(out=t_out[w * 128:(w + 1) * 128, :], in_=of[:])
                if l == 0:
                    nc.gpsimd.collective_compute(
                        "AllGather", mybir.AluOpType.bypass,
                        replica_groups=[list(range(NC))],
                        ins=[cur_slice[:]], outs=[cur_full[:]])
        _f1(); _f2()
    nc.compile()

    in_maps = []
    for k in range(NC):
        in_maps.append({
            "emb_all": emb_all,
            "emb_my": np.ascontiguousarray(emb_all.reshape(NC, SLOTS, D)[k]),
            "idx0": np.ascontiguousarray(idx_in[k]),
            "idx1": np.ascontiguousarray(idx_in[k]),
            "mdl0": np.ascontiguousarray(meta_dl[k]),
            "mvv0": np.ascontiguousarray(meta_vv[k]),
            "mdl1": np.ascontiguousarray(meta_dl[k]),
            "mvv1": np.ascontiguousarray(meta_vv[k]),
            "nz": np.ascontiguousarray(nz_in[k]),
            "w1": w1_in, "b1": b1_in, "w2mu": w2mu_in, "w2lv": w2lv_in,
            "bmu": bmu_in, "blv": blv_in, "iota": iota_np,
        })
    res = run_bass_kernel_spmd(nc, in_maps, core_ids=list(range(NC)))
    outs = np.stack([np.asarray(res.results[k]["outp"]) for k in range(NC)])  # [NC, SLOTS, D]
    final = outs[core_of, slot_of]  # [N, 64]
    return final


def kernel(**inputs):
    final = _build_and_run(inputs)
    n_users = int(inputs['n_users'])
    return final[:n_users], final[n_users:]
